# revision 18
# baseline (speedup 1.0000x reference)
"""Trainium2 Bass kernel for nn_FR_12343736008794.

Fused dual-branch gated conv block:
  xc = cat(x1,x2); x1x = conv1x1(xc,c1); x2x = conv1x1(xc,c2)
  w1 = channel_gate(x1x, x1, m1);  w2 = channel_gate(x2x, x2, m2)
  re1 = w1 + x2; re2 = w2 + x1
  fg1 = spatial_gate(re1, x1) + x2; fg2 = spatial_gate(re2, x2) + x1
  po1 = conv1x1(cat(fg1+FE1, fg2+FE2), p1); po2 = conv1x1(..., p2)

Sharding: pure data-parallel over batch N=32 -> 4 samples per NeuronCore x 8.

Design (v2, bf16):
  - All convs as bf16 PE matmuls (N=512 moving, FWL weight loads).
  - Channel gate: softmax-over-HW via max-of-exp trick (max y on DVE with
    negate, two ACT exps with accum), pooled = t/s via DVE divide.
  - Gate MLP folded host-side to ONE linear layer (w2@w1); sigmoid computed
    as 0.5*tanh(0.5x+0.5b)+0.5 so only the exp/tanh ACT table is ever loaded.
  - Spatial gate without any PE transposes: channel-max via DVE pairwise-max
    tree (512->128) + gpsimd partition_all_reduce(max) (128->1, result
    broadcast to all partitions); channel sums S=sum(q), T=sum(q*re) via
    all-ones [128,128] lhsT matmuls whose outputs are replicated across all
    128 partitions (broadcast for free); V = T/S one DVE divide.
  - fe tensors pre-folded host-side: xf1 = x2+FE1, xf2 = x1+FE2, so
    co = x1*V + xf1 is two tensor ops.
  - p-conv bias and final upcast host-side; outputs shipped bf16.
"""

import sys

sys.path.insert(0, "/opt/trn_rl_repo")

import numpy as np

N_CORES = 8
N, C, H, W = 32, 512, 32, 32
HW = H * W
S = N // N_CORES  # samples per core
NCH = C // 128  # channel chunks of 128
NK = (2 * C) // 128  # contraction k-tiles for the 1024-wide convs

_PROGRAM_CACHE = {}


def build_program(s_per_core=S):
    """Build the per-core Bass program (shared SPMD across 8 cores)."""
    import concourse.bass as bass
    import concourse.mybir as mybir
    import concourse.tile as tile
    from concourse import bacc
    from concourse import bass_isa

    f32 = mybir.dt.float32
    bf16 = mybir.dt.bfloat16
    f16 = mybir.dt.float16
    Alu = mybir.AluOpType
    Act = mybir.ActivationFunctionType
    AX = mybir.AxisListType

    SS = s_per_core
    R = SS * C

    nc = bacc.Bacc("TRN2", target_bir_lowering=False, debug=False)

    dr = {}
    for nm in ("x1", "x2", "xf1", "xf2"):
        dr[nm] = nc.dram_tensor(nm, [R, HW], bf16, kind="ExternalInput").ap()
    for nm in ("c1wT", "c2wT", "p1wT", "p2wT"):
        dr[nm] = nc.dram_tensor(nm, [2 * C, C], bf16, kind="ExternalInput").ap()
    for nm in ("W1T", "W2T"):
        dr[nm] = nc.dram_tensor(nm, [C, C], f16, kind="ExternalInput").ap()
    for nm in ("c1b", "c2b", "gb1", "gb2"):
        dr[nm] = nc.dram_tensor(nm, [C, 1], f32, kind="ExternalInput").ap()
    for nm in ("po1", "po2"):
        dr[nm] = nc.dram_tensor(nm, [R, HW], bf16, kind="ExternalOutput").ap()

    from contextlib import ExitStack

    with tile.TileContext(nc) as tc, ExitStack() as ctx:
        ep = ctx.enter_context
        wpool = ep(tc.tile_pool(name="wpool", bufs=1))
        xpool = ep(tc.tile_pool(name="xpool", bufs=8))
        xfpool = ep(tc.tile_pool(name="xfpool", bufs=5))
        ypool = ep(tc.tile_pool(name="ypool", bufs=2))
        repool = ep(tc.tile_pool(name="repool", bufs=5))
        zpool = ep(tc.tile_pool(name="zpool", bufs=5))
        qpool = ep(tc.tile_pool(name="qpool", bufs=5))
        rpool = ep(tc.tile_pool(name="rpool", bufs=5))
        trpool = ep(tc.tile_pool(name="trpool", bufs=3))
        mbpool = ep(tc.tile_pool(name="mbpool", bufs=2))
        vpool = ep(tc.tile_pool(name="vpool", bufs=4))
        copool = ep(tc.tile_pool(name="copool", bufs=9))
        posb = ep(tc.tile_pool(name="posb", bufs=4))
        smpool = ep(tc.tile_pool(name="smpool", bufs=2))
        stpool = ep(tc.tile_pool(name="stpool", bufs=1))
        xxpool = ep(tc.tile_pool(name="xxpool", bufs=2, space="PSUM"))
        stps = ep(tc.tile_pool(name="stps", bufs=2, space="PSUM"))
        pops = ep(tc.tile_pool(name="pops", bufs=2, space="PSUM"))

        # ---------------- persistent weights / constants ----------------
        cw = {}
        for wnm in ("c1wT", "c2wT", "p1wT", "p2wT"):
            tiles = []
            for kk in range(NK):
                t = wpool.tile([128, C], bf16, name=f"{wnm}_{kk}", tag=f"{wnm}_{kk}")
                nc.sync.dma_start(out=t[:], in_=dr[wnm][kk * 128:(kk + 1) * 128, :])
                tiles.append(t)
            cw[wnm] = tiles
        mw = {}
        for wnm in ("W1T", "W2T"):
            tiles = []
            for kk in range(NCH):
                t = wpool.tile([128, C], f16, name=f"{wnm}_{kk}", tag=f"{wnm}_{kk}")
                nc.sync.dma_start(out=t[:], in_=dr[wnm][kk * 128:(kk + 1) * 128, :])
                tiles.append(t)
            mw[wnm] = tiles
        bias = {}
        for bnm in ("c1b", "c2b", "gb1", "gb2"):
            t = wpool.tile([128, NCH], f32, name=f"b_{bnm}", tag=f"b_{bnm}")
            for kc in range(NCH):
                nc.sync.dma_start(
                    out=t[:, kc:kc + 1], in_=dr[bnm][kc * 128:(kc + 1) * 128, 0:1]
                )
            bias[bnm] = t
        ones = wpool.tile([128, 128], bf16, name="ones", tag="ones")
        nc.vector.memset(ones[:], 1.0)

        # persistent per-sample stats tiles (pooled vec + gates)
        pooled = {
            g: [
                stpool.tile([128, SS], f16, name=f"pooled{g}_{kc}", tag=f"pl{g}{kc}")
                for kc in range(NCH)
            ]
            for g in (1, 2)
        }
        gates = {
            g: [
                stpool.tile([128, SS], f32, name=f"gate{g}_{kc}", tag=f"gt{g}{kc}")
                for kc in range(NCH)
            ]
            for g in (1, 2)
        }

        def emit_loads(n):
            x1t, x2t, xf1t, xf2t = [], [], [], []
            for kc in range(NCH):
                row = slice(n * C + kc * 128, n * C + (kc + 1) * 128)
                t1 = xpool.tile([128, HW], bf16, name=f"x1_{n}_{kc}", tag="x1")
                nc.sync.dma_start(out=t1[:], in_=dr["x1"][row, :])
                x1t.append(t1)
                t2 = xpool.tile([128, HW], bf16, name=f"x2_{n}_{kc}", tag="x2")
                nc.sync.dma_start(out=t2[:], in_=dr["x2"][row, :])
                x2t.append(t2)
            for kc in range(NCH):
                row = slice(n * C + kc * 128, n * C + (kc + 1) * 128)
                f1 = xfpool.tile([128, HW], bf16, name=f"xf1_{n}_{kc}", tag="xf1")
                nc.sync.dma_start(out=f1[:], in_=dr["xf1"][row, :])
                xf1t.append(f1)
                f2 = xfpool.tile([128, HW], bf16, name=f"xf2_{n}_{kc}", tag="xf2")
                nc.sync.dma_start(out=f2[:], in_=dr["xf2"][row, :])
                xf2t.append(f2)
            return x1t, x2t, xf1t, xf2t

        def emit_A(n, x1t, x2t):
            """c-convs + channel-gate stats (softmax-over-HW pooled vecs)."""
            for gidx, (wnm, bnm) in enumerate((("c1wT", "c1b"), ("c2wT", "c2b"))):
                g = gidx + 1
                for kc in range(NCH):
                    xx = xxpool.tile([128, HW], f32, name=f"xx_{n}_{g}_{kc}", tag="xx")
                    for nh in range(2):
                        for kk in range(NK):
                            rhs = (x1t if kk < NCH else x2t)[kk % NCH]
                            nc.tensor.matmul(
                                xx[:, nh * 512:(nh + 1) * 512],
                                cw[wnm][kk][:, kc * 128:(kc + 1) * 128],
                                rhs[:, nh * 512:(nh + 1) * 512],
                                start=(kk == 0),
                                stop=(kk == NK - 1),
                            )
                    y = ypool.tile([128, HW], bf16, name=f"y_{n}_{g}_{kc}", tag="y")
                    nc.scalar.activation(
                        y[:], xx[:], Act.Exp, bias=bias[bnm][:, kc:kc + 1], scale=1.0
                    )
                    nmy = smpool.tile([128, 1], f32, name=f"nmy_{n}_{g}_{kc}", tag="nmy", bufs=3)
                    nc.vector.tensor_reduce(nmy[:], y[:], axis=AX.X, op=Alu.max, negate=True)
                    p = ypool.tile([128, HW], bf16, name=f"p_{n}_{g}_{kc}", tag="p")
                    s = smpool.tile([128, 1], f32, name=f"s_{n}_{g}_{kc}", tag="s", bufs=3)
                    nc.scalar.activation(
                        p[:], y[:], Act.Exp, bias=nmy[:], scale=1.0, accum_out=s[:]
                    )
                    v = ypool.tile([128, HW], bf16, name=f"v_{n}_{g}_{kc}", tag="v")
                    t_ = smpool.tile([128, 1], f32, name=f"t_{n}_{g}_{kc}", tag="t", bufs=3)
                    nc.vector.scalar_tensor_tensor(
                        v[:], p[:], 1.0, xx[:],
                        op0=Alu.mult, op1=Alu.mult, accum_out=t_[:],
                    )
                    rs = smpool.tile([128, 1], f32, name=f"rs_{n}_{g}_{kc}", tag="rs", bufs=3)
                    nc.vector.reciprocal(rs[:], s[:])
                    nc.vector.tensor_scalar(
                        out=pooled[g][kc][:, n:n + 1], in0=t_[:],
                        scalar1=rs[:], scalar2=None, op0=Alu.mult,
                    )

        def emit_B(n):
            """folded gate MLP (1 layer) + exp-form sigmoid."""
            for g, (wnm, gbnm) in ((1, ("W1T", "gb1")), (2, ("W2T", "gb2"))):
                for mt in range(NCH):
                    gp = pops.tile([128, 1], f32, name=f"gp_{n}_{g}_{mt}", tag="pp")
                    for kt in range(NCH):
                        nc.tensor.matmul(
                            gp[:],
                            mw[wnm][kt][:, mt * 128:(mt + 1) * 128],
                            pooled[g][kt][:, n:n + 1],
                            start=(kt == 0),
                            stop=(kt == NCH - 1),
                        )
                    e_ = smpool.tile([128, 1], f32, name=f"e_{n}_{g}_{mt}", tag="e", bufs=3)
                    nc.scalar.activation(
                        e_[:], gp[:], Act.Exp, bias=bias[gbnm][:, mt:mt + 1], scale=-1.0
                    )
                    ge = smpool.tile([128, 1], f32, name=f"ge_{n}_{g}_{mt}", tag="ge", bufs=3)
                    nc.vector.tensor_scalar_add(ge[:], e_[:], 1.0)
                    nc.vector.reciprocal(gates[g][mt][:, n:n + 1], ge[:])

        def emit_D(n, x1t, x2t):
            """re build + spatial-gate softmax: V = sum(q*re)/sum(q)."""
            Vt = {}
            for t in (1, 2):
                xa = x1t if t == 1 else x2t
                xb = x2t if t == 1 else x1t
                ret, zt = [], []
                for kc in range(NCH):
                    xg = repool.tile([128, HW], bf16, name=f"xg_{n}_{t}_{kc}", tag="xg", bufs=2)
                    nc.vector.tensor_scalar_mul(xg[:], xa[kc][:], gates[t][kc][:, n:n + 1])
                    rh = repool.tile([128, HW], bf16, name=f"re_{n}_{t}_{kc}", tag="re")
                    nc.vector.tensor_tensor(rh[:], xg[:], xb[kc][:], Alu.add)
                    ret.append(rh)
                    zh = zpool.tile([128, HW], bf16, name=f"z_{n}_{t}_{kc}", tag="z")
                    nc.scalar.activation(zh[:], rh[:], Act.Exp)
                    zt.append(zh)
                # channel max: pairwise tree 512->128, then cross-partition max
                m01 = trpool.tile([128, HW], bf16, name=f"m01_{n}_{t}", tag="tr")
                nc.vector.tensor_tensor(m01[:], zt[0][:], zt[1][:], Alu.max)
                m23 = trpool.tile([128, HW], bf16, name=f"m23_{n}_{t}", tag="tr")
                nc.vector.tensor_tensor(m23[:], zt[2][:], zt[3][:], Alu.max)
                m1 = trpool.tile([128, HW], bf16, name=f"m1_{n}_{t}", tag="tr")
                nc.vector.tensor_tensor(m1[:], m01[:], m23[:], Alu.max)
                mb = mbpool.tile([128, HW], bf16, name=f"mb_{n}_{t}", tag="mb")
                nc.gpsimd.partition_all_reduce(
                    mb[:], m1[:], 128, bass_isa.ReduceOp.max
                )
                qt, rt = [], []
                for kc in range(NCH):
                    w_ = zpool.tile([128, HW], bf16, name=f"w_{n}_{t}_{kc}", tag="w", bufs=2)
                    nc.vector.tensor_tensor(w_[:], zt[kc][:], mb[:], Alu.subtract)
                    qh = qpool.tile([128, HW], bf16, name=f"q_{n}_{t}_{kc}", tag="q")
                    nc.scalar.activation(qh[:], w_[:], Act.Exp)
                    qt.append(qh)
                    rh2 = rpool.tile([128, HW], bf16, name=f"r_{n}_{t}_{kc}", tag="r")
                    nc.vector.tensor_tensor(rh2[:], qh[:], ret[kc][:], Alu.mult)
                    rt.append(rh2)
                # S = sum_c q, T = sum_c q*re via all-ones lhsT (output rows
                # replicated across all 128 partitions -> broadcast for free)
                vh = vpool.tile([128, HW], bf16, name=f"v_{n}_{t}", tag="vv", bufs=2)
                for nh in range(2):
                    sl = slice(nh * 512, (nh + 1) * 512)
                    sf = stps.tile([128, 512], f32, name=f"sf_{n}_{t}_{nh}", tag="st")
                    for kc in range(NCH):
                        nc.tensor.matmul(
                            sf[:], ones[:], qt[kc][:, sl],
                            start=(kc == 0), stop=(kc == NCH - 1),
                        )
                    tf = stps.tile([128, 512], f32, name=f"tf_{n}_{t}_{nh}", tag="st")
                    for kc in range(NCH):
                        nc.tensor.matmul(
                            tf[:], ones[:], rt[kc][:, sl],
                            start=(kc == 0), stop=(kc == NCH - 1),
                        )
                    # V = T * (1/S); reciprocal on DVE (an ACT ln/exp pair
                    # would thrash the activation table set against Exp)
                    rsf = vpool.tile([128, 512], f32, name=f"rsf_{n}_{t}_{nh}", tag="rsf", bufs=2)
                    nc.vector.reciprocal(rsf[:], sf[:])
                    nc.vector.tensor_tensor(vh[:, sl], tf[:], rsf[:], Alu.mult)
                Vt[t] = vh
            return Vt

        def emit_co(n, x1t, x2t, xf1t, xf2t, Vt):
            """co = x1*V + (x2 + fe) with xf = x2+fe folded host-side."""
            co = {1: [], 2: []}
            for t in (1, 2):
                xa = x1t if t == 1 else x2t
                xf = xf1t if t == 1 else xf2t
                for kc in range(NCH):
                    tt = copool.tile([128, HW], bf16, name=f"ct_{n}_{t}_{kc}", tag="ct", bufs=2)
                    nc.vector.tensor_tensor(tt[:], xa[kc][:], Vt[t][:], Alu.mult)
                    coh = copool.tile([128, HW], bf16, name=f"co_{n}_{t}_{kc}", tag="co")
                    eng = nc.gpsimd if (kc >= 2) else nc.vector
                    eng.tensor_tensor(coh[:], tt[:], xf[kc][:], Alu.add)
                    co[t].append(coh)
            return co

        def emit_F(n, co):
            """p-convs + PSUM->SBUF copies + output DMA."""
            for nh in range(2):
                sl = slice(nh * 512, (nh + 1) * 512)
                for pc, (wnm, onm) in enumerate((("p1wT", "po1"), ("p2wT", "po2"))):
                    for km in range(NCH):
                        po = pops.tile([128, 512], f32, name=f"po_{n}_{pc}_{nh}_{km}", tag="pp")
                        for kk in range(NK):
                            rhs = co[1 if kk < NCH else 2][kk % NCH]
                            nc.tensor.matmul(
                                po[:],
                                cw[wnm][kk][:, km * 128:(km + 1) * 128],
                                rhs[:, sl],
                                start=(kk == 0),
                                stop=(kk == NK - 1),
                            )
                        ps = posb.tile([128, 512], bf16, name=f"ps_{n}_{pc}_{nh}_{km}", tag="ps")
                        nc.scalar.copy(ps[:], po[:])
                        nc.sync.dma_start(
                            out=dr[onm][n * C + km * 128: n * C + (km + 1) * 128, sl],
                            in_=ps[:],
                        )

        # Software-pipelined emission: every engine queue is in-order, so the
        # previous sample's dependency-stalled tail (co build + p-convs) is
        # emitted BEHIND the next sample's conv matmuls - the PE never sits
        # behind an elementwise chain it does not feed.
        pend = None
        for n in range(SS):
            x1t, x2t, xf1t, xf2t = emit_loads(n)
            emit_A(n, x1t, x2t)
            if pend is not None:
                pco = emit_co(*pend)
            emit_B(n)
            if pend is not None:
                emit_F(pend[0], pco)
            Vt = emit_D(n, x1t, x2t)
            pend = (n, x1t, x2t, xf1t, xf2t, Vt)
        pco = emit_co(*pend)
        emit_F(pend[0], pco)
    nc.compile()
    return nc


def _host_prep(inputs, s_per_core=S, n_cores=N_CORES):
    """Build per-core input maps (host-side folds, bf16 casts)."""
    import ml_dtypes

    f = np.float32
    bf = ml_dtypes.bfloat16
    x1 = np.ascontiguousarray(inputs["x1"], dtype=f).reshape(N, C, HW)
    x2 = np.ascontiguousarray(inputs["x2"], dtype=f).reshape(N, C, HW)
    fe1 = np.ascontiguousarray(inputs["FE_x1"], dtype=f).reshape(N, C, HW)
    fe2 = np.ascontiguousarray(inputs["FE_x2"], dtype=f).reshape(N, C, HW)
    xf1 = (x2 + fe1).astype(bf)
    xf2 = (x1 + fe2).astype(bf)
    x1b = x1.astype(bf)
    x2b = x2.astype(bf)

    wT = {
        "c1wT": np.ascontiguousarray(inputs["c1_w"].astype(f).T).astype(bf),
        "c2wT": np.ascontiguousarray(inputs["c2_w"].astype(f).T).astype(bf),
        "p1wT": np.ascontiguousarray(inputs["p1_w"].astype(f).T).astype(bf),
        "p2wT": np.ascontiguousarray(inputs["p2_w"].astype(f).T).astype(bf),
    }
    # fold the two gate-MLP layers into one: g = W@pooled_nb + b_all
    # (pooled_nb excludes the conv bias; it is folded into b_all)
    W1 = inputs["m1_w2"].astype(np.float64) @ inputs["m1_w1"].astype(np.float64)
    W2 = inputs["m2_w2"].astype(np.float64) @ inputs["m2_w1"].astype(np.float64)
    b1 = (
        W1 @ inputs["c1_b"].astype(np.float64)
        + inputs["m1_w2"].astype(np.float64) @ inputs["m1_b1"].astype(np.float64)
        + inputs["m1_b2"].astype(np.float64)
    )
    b2 = (
        W2 @ inputs["c2_b"].astype(np.float64)
        + inputs["m2_w2"].astype(np.float64) @ inputs["m2_b1"].astype(np.float64)
        + inputs["m2_b2"].astype(np.float64)
    )
    mwT = {
        "W1T": np.ascontiguousarray(W1.T).astype(np.float16),
        "W2T": np.ascontiguousarray(W2.T).astype(np.float16),
    }
    vecs = {
        "c1b": inputs["c1_b"].astype(f),
        "c2b": inputs["c2_b"].astype(f),
        "gb1": (-b1).astype(f),
        "gb2": (-b2).astype(f),
    }

    in_maps = []
    for c in range(n_cores):
        slc = slice(c * s_per_core, (c + 1) * s_per_core)
        m = {
            "x1": x1b[slc].reshape(s_per_core * C, HW),
            "x2": x2b[slc].reshape(s_per_core * C, HW),
            "xf1": xf1[slc].reshape(s_per_core * C, HW),
            "xf2": xf2[slc].reshape(s_per_core * C, HW),
        }
        for k, v in wT.items():
            m[k] = v
        for k, v in mwT.items():
            m[k] = v
        for k, v in vecs.items():
            m[k] = v.reshape(C, 1)
        in_maps.append(m)
    return in_maps


def kernel(**inputs):
    from concourse.bass_utils import run_bass_kernel_spmd

    key = "prog"
    if key not in _PROGRAM_CACHE:
        _PROGRAM_CACHE[key] = build_program()
    nc = _PROGRAM_CACHE[key]

    in_maps = _host_prep(inputs)
    res = run_bass_kernel_spmd(nc, in_maps, core_ids=list(range(N_CORES)))

    po1 = np.concatenate(
        [np.asarray(r["po1"], dtype=np.float32).reshape(S, C, HW) for r in res.results],
        axis=0,
    ).reshape(N, C, H, W)
    po2 = np.concatenate(
        [np.asarray(r["po2"], dtype=np.float32).reshape(S, C, HW) for r in res.results],
        axis=0,
    ).reshape(N, C, H, W)
    # p-conv biases applied host-side (exact)
    po1 = po1 + inputs["p1_b"].astype(np.float32)[None, :, None, None]
    po2 = po2 + inputs["p2_b"].astype(np.float32)[None, :, None, None]
    return po1, po2


# revision 20
# speedup vs baseline: 1.0718x; 1.0718x over previous
"""Trainium2 Bass kernel for nn_FR_12343736008794.

Fused dual-branch gated conv block:
  xc = cat(x1,x2); x1x = conv1x1(xc,c1); x2x = conv1x1(xc,c2)
  w1 = channel_gate(x1x, x1, m1);  w2 = channel_gate(x2x, x2, m2)
  re1 = w1 + x2; re2 = w2 + x1
  fg1 = spatial_gate(re1, x1) + x2; fg2 = spatial_gate(re2, x2) + x1
  po1 = conv1x1(cat(fg1+FE1, fg2+FE2), p1); po2 = conv1x1(..., p2)

Sharding: pure data-parallel over batch N=32 -> 4 samples per NeuronCore x 8.

Design (v2, bf16):
  - All convs as bf16 PE matmuls (N=512 moving, FWL weight loads).
  - Channel gate: softmax-over-HW via max-of-exp trick (max y on DVE with
    negate, two ACT exps with accum), pooled = t/s via DVE divide.
  - Gate MLP folded host-side to ONE linear layer (w2@w1); sigmoid computed
    as 0.5*tanh(0.5x+0.5b)+0.5 so only the exp/tanh ACT table is ever loaded.
  - Spatial gate without any PE transposes: channel-max via DVE pairwise-max
    tree (512->128) + gpsimd partition_all_reduce(max) (128->1, result
    broadcast to all partitions); channel sums S=sum(q), T=sum(q*re) via
    all-ones [128,128] lhsT matmuls whose outputs are replicated across all
    128 partitions (broadcast for free); V = T/S one DVE divide.
  - fe tensors pre-folded host-side: xf1 = x2+FE1, xf2 = x1+FE2, so
    co = x1*V + xf1 is two tensor ops.
  - p-conv bias and final upcast host-side; outputs shipped bf16.
"""

import sys

sys.path.insert(0, "/opt/trn_rl_repo")

import numpy as np

N_CORES = 8
N, C, H, W = 32, 512, 32, 32
HW = H * W
S = N // N_CORES  # samples per core
NCH = C // 128  # channel chunks of 128
NK = (2 * C) // 128  # contraction k-tiles for the 1024-wide convs

_PROGRAM_CACHE = {}


def build_program(s_per_core=S):
    """Build the per-core Bass program (shared SPMD across 8 cores)."""
    import concourse.bass as bass
    import concourse.mybir as mybir
    import concourse.tile as tile
    from concourse import bacc
    from concourse import bass_isa

    f32 = mybir.dt.float32
    bf16 = mybir.dt.bfloat16
    f16 = mybir.dt.float16
    Alu = mybir.AluOpType
    Act = mybir.ActivationFunctionType
    AX = mybir.AxisListType

    SS = s_per_core
    R = SS * C

    nc = bacc.Bacc("TRN2", target_bir_lowering=False, debug=False)

    dr = {}
    for nm in ("x1", "x2", "xf1", "xf2"):
        dr[nm] = nc.dram_tensor(nm, [R, HW], bf16, kind="ExternalInput").ap()
    for nm in ("c1wT", "c2wT", "p1wT", "p2wT"):
        dr[nm] = nc.dram_tensor(nm, [2 * C, C], bf16, kind="ExternalInput").ap()
    for nm in ("W1T", "W2T"):
        dr[nm] = nc.dram_tensor(nm, [C, C], f16, kind="ExternalInput").ap()
    for nm in ("c1b", "c2b", "gb1", "gb2"):
        dr[nm] = nc.dram_tensor(nm, [C, 1], f32, kind="ExternalInput").ap()
    for nm in ("po1", "po2"):
        dr[nm] = nc.dram_tensor(nm, [R, HW], bf16, kind="ExternalOutput").ap()

    from contextlib import ExitStack

    with tile.TileContext(nc) as tc, ExitStack() as ctx:
        ep = ctx.enter_context
        wpool = ep(tc.tile_pool(name="wpool", bufs=1))
        xpool = ep(tc.tile_pool(name="xpool", bufs=8))
        xfpool = ep(tc.tile_pool(name="xfpool", bufs=4))
        ypool = ep(tc.tile_pool(name="ypool", bufs=2))
        repool = ep(tc.tile_pool(name="repool", bufs=5))
        zpool = ep(tc.tile_pool(name="zpool", bufs=5))
        qpool = ep(tc.tile_pool(name="qpool", bufs=9))
        rpool = ep(tc.tile_pool(name="rpool", bufs=8))
        trpool = ep(tc.tile_pool(name="trpool", bufs=3))
        mbpool = ep(tc.tile_pool(name="mbpool", bufs=2))
        vpool = ep(tc.tile_pool(name="vpool", bufs=4))
        copool = ep(tc.tile_pool(name="copool", bufs=8))
        posb = ep(tc.tile_pool(name="posb", bufs=3))
        smpool = ep(tc.tile_pool(name="smpool", bufs=2))
        stpool = ep(tc.tile_pool(name="stpool", bufs=1))
        xxpool = ep(tc.tile_pool(name="xxpool", bufs=2, space="PSUM"))
        stps = ep(tc.tile_pool(name="stps", bufs=2, space="PSUM"))
        pops = ep(tc.tile_pool(name="pops", bufs=2, space="PSUM"))

        # ---------------- persistent weights / constants ----------------
        cw = {}
        for wnm in ("c1wT", "c2wT", "p1wT", "p2wT"):
            tiles = []
            for kk in range(NK):
                t = wpool.tile([128, C], bf16, name=f"{wnm}_{kk}", tag=f"{wnm}_{kk}")
                nc.sync.dma_start(out=t[:], in_=dr[wnm][kk * 128:(kk + 1) * 128, :])
                tiles.append(t)
            cw[wnm] = tiles
        mw = {}
        for wnm in ("W1T", "W2T"):
            tiles = []
            for kk in range(NCH):
                t = wpool.tile([128, C], f16, name=f"{wnm}_{kk}", tag=f"{wnm}_{kk}")
                nc.sync.dma_start(out=t[:], in_=dr[wnm][kk * 128:(kk + 1) * 128, :])
                tiles.append(t)
            mw[wnm] = tiles
        bias = {}
        for bnm in ("c1b", "c2b", "gb1", "gb2"):
            t = wpool.tile([128, NCH], f32, name=f"b_{bnm}", tag=f"b_{bnm}")
            for kc in range(NCH):
                nc.sync.dma_start(
                    out=t[:, kc:kc + 1], in_=dr[bnm][kc * 128:(kc + 1) * 128, 0:1]
                )
            bias[bnm] = t
        ones = wpool.tile([128, 128], bf16, name="ones", tag="ones")
        nc.vector.memset(ones[:], 1.0)

        # persistent per-sample stats tiles (pooled vec + gates)
        pooled = {
            g: [
                stpool.tile([128, SS], f16, name=f"pooled{g}_{kc}", tag=f"pl{g}{kc}")
                for kc in range(NCH)
            ]
            for g in (1, 2)
        }
        gates = {
            g: [
                stpool.tile([128, SS], f32, name=f"gate{g}_{kc}", tag=f"gt{g}{kc}")
                for kc in range(NCH)
            ]
            for g in (1, 2)
        }

        def emit_loads(n):
            x1t, x2t, xf1t, xf2t = [], [], [], []
            for kc in range(NCH):
                row = slice(n * C + kc * 128, n * C + (kc + 1) * 128)
                t1 = xpool.tile([128, HW], bf16, name=f"x1_{n}_{kc}", tag="x1")
                nc.sync.dma_start(out=t1[:], in_=dr["x1"][row, :])
                x1t.append(t1)
                t2 = xpool.tile([128, HW], bf16, name=f"x2_{n}_{kc}", tag="x2")
                nc.sync.dma_start(out=t2[:], in_=dr["x2"][row, :])
                x2t.append(t2)
            for kc in range(NCH):
                row = slice(n * C + kc * 128, n * C + (kc + 1) * 128)
                f1 = xfpool.tile([128, HW], bf16, name=f"xf1_{n}_{kc}", tag="xf1")
                nc.sync.dma_start(out=f1[:], in_=dr["xf1"][row, :])
                xf1t.append(f1)
                f2 = xfpool.tile([128, HW], bf16, name=f"xf2_{n}_{kc}", tag="xf2")
                nc.sync.dma_start(out=f2[:], in_=dr["xf2"][row, :])
                xf2t.append(f2)
            return x1t, x2t, xf1t, xf2t

        def emit_A(n, x1t, x2t):
            """c-convs + channel-gate stats (softmax-over-HW pooled vecs)."""
            for gidx, (wnm, bnm) in enumerate((("c1wT", "c1b"), ("c2wT", "c2b"))):
                g = gidx + 1
                for kc in range(NCH):
                    xx = xxpool.tile([128, HW], f32, name=f"xx_{n}_{g}_{kc}", tag="xx")
                    for nh in range(2):
                        for kk in range(NK):
                            rhs = (x1t if kk < NCH else x2t)[kk % NCH]
                            nc.tensor.matmul(
                                xx[:, nh * 512:(nh + 1) * 512],
                                cw[wnm][kk][:, kc * 128:(kc + 1) * 128],
                                rhs[:, nh * 512:(nh + 1) * 512],
                                start=(kk == 0),
                                stop=(kk == NK - 1),
                            )
                    y = ypool.tile([128, HW], bf16, name=f"y_{n}_{g}_{kc}", tag="y")
                    nc.scalar.activation(
                        y[:], xx[:], Act.Exp, bias=bias[bnm][:, kc:kc + 1], scale=1.0
                    )
                    nmy = smpool.tile([128, 1], f32, name=f"nmy_{n}_{g}_{kc}", tag="nmy", bufs=3)
                    nc.vector.tensor_reduce(nmy[:], y[:], axis=AX.X, op=Alu.max, negate=True)
                    p = ypool.tile([128, HW], bf16, name=f"p_{n}_{g}_{kc}", tag="p")
                    s = smpool.tile([128, 1], f32, name=f"s_{n}_{g}_{kc}", tag="s", bufs=3)
                    nc.scalar.activation(
                        p[:], y[:], Act.Exp, bias=nmy[:], scale=1.0, accum_out=s[:]
                    )
                    v = ypool.tile([128, HW], bf16, name=f"v_{n}_{g}_{kc}", tag="v")
                    t_ = smpool.tile([128, 1], f32, name=f"t_{n}_{g}_{kc}", tag="t", bufs=3)
                    nc.vector.scalar_tensor_tensor(
                        v[:], p[:], 1.0, xx[:],
                        op0=Alu.mult, op1=Alu.mult, accum_out=t_[:],
                    )
                    rs = smpool.tile([128, 1], f32, name=f"rs_{n}_{g}_{kc}", tag="rs", bufs=3)
                    nc.vector.reciprocal(rs[:], s[:])
                    nc.vector.tensor_scalar(
                        out=pooled[g][kc][:, n:n + 1], in0=t_[:],
                        scalar1=rs[:], scalar2=None, op0=Alu.mult,
                    )

        def emit_B(n):
            """folded gate MLP (1 layer) + exp-form sigmoid."""
            for g, (wnm, gbnm) in ((1, ("W1T", "gb1")), (2, ("W2T", "gb2"))):
                for mt in range(NCH):
                    gp = pops.tile([128, 1], f32, name=f"gp_{n}_{g}_{mt}", tag="pp")
                    for kt in range(NCH):
                        nc.tensor.matmul(
                            gp[:],
                            mw[wnm][kt][:, mt * 128:(mt + 1) * 128],
                            pooled[g][kt][:, n:n + 1],
                            start=(kt == 0),
                            stop=(kt == NCH - 1),
                        )
                    e_ = smpool.tile([128, 1], f32, name=f"e_{n}_{g}_{mt}", tag="e", bufs=3)
                    nc.scalar.activation(
                        e_[:], gp[:], Act.Exp, bias=bias[gbnm][:, mt:mt + 1], scale=-1.0
                    )
                    ge = smpool.tile([128, 1], f32, name=f"ge_{n}_{g}_{mt}", tag="ge", bufs=3)
                    nc.vector.tensor_scalar_add(ge[:], e_[:], 1.0)
                    nc.vector.reciprocal(gates[g][mt][:, n:n + 1], ge[:])

        def emit_Delems(n, t, x1t, x2t):
            """re build + z=exp(re) + channel-max + q,r for one branch."""
            xa = x1t if t == 1 else x2t
            xb = x2t if t == 1 else x1t
            ret, zt = [], []
            for kc in range(NCH):
                xg = repool.tile([128, HW], bf16, name=f"xg_{n}_{t}_{kc}", tag="xg", bufs=2)
                nc.vector.tensor_scalar_mul(xg[:], xa[kc][:], gates[t][kc][:, n:n + 1])
                rh = repool.tile([128, HW], bf16, name=f"re_{n}_{t}_{kc}", tag="re")
                nc.vector.tensor_tensor(rh[:], xg[:], xb[kc][:], Alu.add)
                ret.append(rh)
                zh = zpool.tile([128, HW], bf16, name=f"z_{n}_{t}_{kc}", tag="z")
                nc.scalar.activation(zh[:], rh[:], Act.Exp)
                zt.append(zh)
            # channel max: pairwise tree 512->128, then cross-partition max
            m01 = trpool.tile([128, HW], bf16, name=f"m01_{n}_{t}", tag="tr")
            nc.vector.tensor_tensor(m01[:], zt[0][:], zt[1][:], Alu.max)
            m23 = trpool.tile([128, HW], bf16, name=f"m23_{n}_{t}", tag="tr")
            nc.vector.tensor_tensor(m23[:], zt[2][:], zt[3][:], Alu.max)
            m1 = trpool.tile([128, HW], bf16, name=f"m1_{n}_{t}", tag="tr")
            nc.vector.tensor_tensor(m1[:], m01[:], m23[:], Alu.max)
            mb = mbpool.tile([128, HW], bf16, name=f"mb_{n}_{t}", tag="mb")
            nc.gpsimd.partition_all_reduce(
                mb[:], m1[:], 128, bass_isa.ReduceOp.max
            )
            qt, rt = [], []
            for kc in range(NCH):
                w_ = zpool.tile([128, HW], bf16, name=f"w_{n}_{t}_{kc}", tag="w", bufs=2)
                nc.vector.tensor_tensor(w_[:], zt[kc][:], mb[:], Alu.subtract)
                qh = qpool.tile([128, HW], bf16, name=f"q_{n}_{t}_{kc}", tag="q")
                nc.scalar.activation(qh[:], w_[:], Act.Exp)
                qt.append(qh)
                rh2 = rpool.tile([128, HW], bf16, name=f"r_{n}_{t}_{kc}", tag="r")
                nc.vector.tensor_tensor(rh2[:], qh[:], ret[kc][:], Alu.mult)
                rt.append(rh2)
            return qt, rt

        def emit_STV(n, t, qt, rt):
            """S = sum_c q, T = sum_c q*re via all-ones lhsT (output rows
            replicated across all 128 partitions -> broadcast for free);
            V = T/S with 1/S as exp(-ln S) on the Scalar engine."""
            vh = vpool.tile([128, HW], bf16, name=f"v_{n}_{t}", tag="vv", bufs=2)
            for nh in range(2):
                sl = slice(nh * 512, (nh + 1) * 512)
                sf = stps.tile([128, 512], f32, name=f"sf_{n}_{t}_{nh}", tag="st")
                for kc in range(NCH):
                    nc.tensor.matmul(
                        sf[:], ones[:], qt[kc][:, sl],
                        start=(kc == 0), stop=(kc == NCH - 1),
                    )
                tf = stps.tile([128, 512], f32, name=f"tf_{n}_{t}_{nh}", tag="st")
                for kc in range(NCH):
                    nc.tensor.matmul(
                        tf[:], ones[:], rt[kc][:, sl],
                        start=(kc == 0), stop=(kc == NCH - 1),
                    )
                lnS = vpool.tile([128, 512], f32, name=f"lnS_{n}_{t}_{nh}", tag="lnS", bufs=2)
                nc.scalar.activation(lnS[:], sf[:], Act.Ln)
                rsf = vpool.tile([128, 512], bf16, name=f"rsf_{n}_{t}_{nh}", tag="rsf", bufs=2)
                nc.scalar.activation(rsf[:], lnS[:], Act.Exp, scale=-1.0)
                nc.vector.tensor_tensor(vh[:, sl], tf[:], rsf[:], Alu.mult)
            return vh

        def emit_co(n, x1t, x2t, xf1t, xf2t, Vt):
            """co = x1*V + (x2 + fe) with xf = x2+fe folded host-side."""
            co = {1: [], 2: []}
            for t in (1, 2):
                xa = x1t if t == 1 else x2t
                xf = xf1t if t == 1 else xf2t
                for kc in range(NCH):
                    tt = copool.tile([128, HW], bf16, name=f"ct_{n}_{t}_{kc}", tag="ct", bufs=2)
                    nc.vector.tensor_tensor(tt[:], xa[kc][:], Vt[t][:], Alu.mult)
                    coh = copool.tile([128, HW], bf16, name=f"co_{n}_{t}_{kc}", tag="co")
                    eng = nc.gpsimd if (kc >= 2) else nc.vector
                    eng.tensor_tensor(coh[:], tt[:], xf[kc][:], Alu.add)
                    co[t].append(coh)
            return co

        def emit_F_half(n, co, nh):
            """p-convs + PSUM->SBUF copies + output DMA for one spatial half."""
            sl = slice(nh * 512, (nh + 1) * 512)
            for pc, (wnm, onm) in enumerate((("p1wT", "po1"), ("p2wT", "po2"))):
                for km in range(NCH):
                    po = pops.tile([128, 512], f32, name=f"po_{n}_{pc}_{nh}_{km}", tag="pp")
                    for kk in range(NK):
                        rhs = co[1 if kk < NCH else 2][kk % NCH]
                        nc.tensor.matmul(
                            po[:],
                            cw[wnm][kk][:, km * 128:(km + 1) * 128],
                            rhs[:, sl],
                            start=(kk == 0),
                            stop=(kk == NK - 1),
                        )
                    ps = posb.tile([128, 512], bf16, name=f"ps_{n}_{pc}_{nh}_{km}", tag="ps")
                    if km == 3:
                        nc.vector.tensor_copy(ps[:], po[:])
                    else:
                        nc.scalar.copy(ps[:], po[:])
                    nc.sync.dma_start(
                        out=dr[onm][n * C + km * 128: n * C + (km + 1) * 128, sl],
                        in_=ps[:],
                    )

        # Software-pipelined emission: every engine queue is in-order.
        # Per iteration the PE queue is [A(n), B(n), F1(n-1), F2(n-1),
        # ST(n)]; the ACT queue sees branch-1 z/q exps before the po copies
        # so it never idles at the copy barrier; the previous sample's
        # p-convs fill the PE while this sample's softmax chains run.
        pend = None
        for n in range(SS):
            x1t, x2t, xf1t, xf2t = emit_loads(n)
            emit_A(n, x1t, x2t)
            if pend is not None:
                pco = emit_co(*pend)
            emit_B(n)
            d1 = emit_Delems(n, 1, x1t, x2t)
            if pend is not None:
                emit_F_half(pend[0], pco, 0)
            d2 = emit_Delems(n, 2, x1t, x2t)
            if pend is not None:
                emit_F_half(pend[0], pco, 1)
            Vt = {1: emit_STV(n, 1, *d1), 2: emit_STV(n, 2, *d2)}
            pend = (n, x1t, x2t, xf1t, xf2t, Vt)
        pco = emit_co(*pend)
        emit_F_half(pend[0], pco, 0)
        emit_F_half(pend[0], pco, 1)
    nc.compile()
    return nc


def _host_prep(inputs, s_per_core=S, n_cores=N_CORES):
    """Build per-core input maps (host-side folds, bf16 casts)."""
    import ml_dtypes

    f = np.float32
    bf = ml_dtypes.bfloat16
    x1 = np.ascontiguousarray(inputs["x1"], dtype=f).reshape(N, C, HW)
    x2 = np.ascontiguousarray(inputs["x2"], dtype=f).reshape(N, C, HW)
    fe1 = np.ascontiguousarray(inputs["FE_x1"], dtype=f).reshape(N, C, HW)
    fe2 = np.ascontiguousarray(inputs["FE_x2"], dtype=f).reshape(N, C, HW)
    xf1 = (x2 + fe1).astype(bf)
    xf2 = (x1 + fe2).astype(bf)
    x1b = x1.astype(bf)
    x2b = x2.astype(bf)

    wT = {
        "c1wT": np.ascontiguousarray(inputs["c1_w"].astype(f).T).astype(bf),
        "c2wT": np.ascontiguousarray(inputs["c2_w"].astype(f).T).astype(bf),
        "p1wT": np.ascontiguousarray(inputs["p1_w"].astype(f).T).astype(bf),
        "p2wT": np.ascontiguousarray(inputs["p2_w"].astype(f).T).astype(bf),
    }
    # fold the two gate-MLP layers into one: g = W@pooled_nb + b_all
    # (pooled_nb excludes the conv bias; it is folded into b_all)
    W1 = inputs["m1_w2"].astype(np.float64) @ inputs["m1_w1"].astype(np.float64)
    W2 = inputs["m2_w2"].astype(np.float64) @ inputs["m2_w1"].astype(np.float64)
    b1 = (
        W1 @ inputs["c1_b"].astype(np.float64)
        + inputs["m1_w2"].astype(np.float64) @ inputs["m1_b1"].astype(np.float64)
        + inputs["m1_b2"].astype(np.float64)
    )
    b2 = (
        W2 @ inputs["c2_b"].astype(np.float64)
        + inputs["m2_w2"].astype(np.float64) @ inputs["m2_b1"].astype(np.float64)
        + inputs["m2_b2"].astype(np.float64)
    )
    mwT = {
        "W1T": np.ascontiguousarray(W1.T).astype(np.float16),
        "W2T": np.ascontiguousarray(W2.T).astype(np.float16),
    }
    vecs = {
        "c1b": inputs["c1_b"].astype(f),
        "c2b": inputs["c2_b"].astype(f),
        "gb1": (-b1).astype(f),
        "gb2": (-b2).astype(f),
    }

    in_maps = []
    for c in range(n_cores):
        slc = slice(c * s_per_core, (c + 1) * s_per_core)
        m = {
            "x1": x1b[slc].reshape(s_per_core * C, HW),
            "x2": x2b[slc].reshape(s_per_core * C, HW),
            "xf1": xf1[slc].reshape(s_per_core * C, HW),
            "xf2": xf2[slc].reshape(s_per_core * C, HW),
        }
        for k, v in wT.items():
            m[k] = v
        for k, v in mwT.items():
            m[k] = v
        for k, v in vecs.items():
            m[k] = v.reshape(C, 1)
        in_maps.append(m)
    return in_maps


def kernel(**inputs):
    from concourse.bass_utils import run_bass_kernel_spmd

    key = "prog"
    if key not in _PROGRAM_CACHE:
        _PROGRAM_CACHE[key] = build_program()
    nc = _PROGRAM_CACHE[key]

    in_maps = _host_prep(inputs)
    res = run_bass_kernel_spmd(nc, in_maps, core_ids=list(range(N_CORES)))

    po1 = np.concatenate(
        [np.asarray(r["po1"], dtype=np.float32).reshape(S, C, HW) for r in res.results],
        axis=0,
    ).reshape(N, C, H, W)
    po2 = np.concatenate(
        [np.asarray(r["po2"], dtype=np.float32).reshape(S, C, HW) for r in res.results],
        axis=0,
    ).reshape(N, C, H, W)
    # p-conv biases applied host-side (exact)
    po1 = po1 + inputs["p1_b"].astype(np.float32)[None, :, None, None]
    po2 = po2 + inputs["p2_b"].astype(np.float32)[None, :, None, None]
    return po1, po2
